# revision 1
# baseline (speedup 1.0000x reference)
"""CARAFE upsampling kernel for 8 Trainium2 NeuronCores.

Reference op (per batch b):
  xc   = conv1x1(x, w1) + b1                     # (CC=64, H, W)
  mask = conv3x3(xc, w2, pad=1) + b2             # (100, H, W)
  mask = softmax over the 25 kernel taps (per q in 4 = SF*SF groups)
  out[q, c, h, w] = sum_k mask[q, k, h, w] * x[c, h+di-2, w+dj-2]
  out pixel-shuffled by SF=2 -> (C, 2H, 2W)

Sharding: 8 shards = batch(4) x H-halves(2). Each core gets a padded
x slice [256, 36, 68] (2 halo rows / 2 zero-pad cols on each side) and
produces out rows [32 rows x 64 cols x 4 quadrants x 256 ch]; the host
performs the pixel shuffle + concat.
"""

import os
from functools import lru_cache

import numpy as np

import concourse.bass as bass
import concourse.mybir as mybir
from concourse import bacc
import concourse.tile as tile
from concourse.bass_utils import run_bass_kernel_spmd

F32 = mybir.dt.float32
BF16 = mybir.dt.bfloat16
import ml_dtypes as _mld

_BF16NP = _mld.bfloat16

# Problem constants (hardcoded; kernel.py must be self-contained).
B, C, H, W = 4, 256, 64, 64
CC = 64           # compressed channels
SF = 2            # scale factor
K5 = 5            # up-kernel
KA = K5 * K5      # 25 taps
NQ = SF * SF      # 4 quadrants
NM = NQ * KA      # 100 mask channels

HL = 32           # local (per-shard) output rows
HP = HL + 4       # padded rows
WP = W + 4        # padded cols
NPIX = HL * W     # 2048 output pixels per shard
NPADPIX = HP * WP # 2448 padded pixels

N_CORES = 8


def _build_program(trace_debug: bool = False):
    """Build the SPMD Bass program (identical on all cores)."""
    nc = bacc.Bacc("TRN2", target_bir_lowering=False, debug=False)

    # ---- DRAM parameters -------------------------------------------------
    x0_d = nc.dram_tensor("x0", [128, HP, WP], F32, kind="ExternalInput")
    x1_d = nc.dram_tensor("x1", [128, HP, WP], F32, kind="ExternalInput")
    w1t_d = nc.dram_tensor("w1t", [2, 128, CC], F32, kind="ExternalInput")
    w2t_d = nc.dram_tensor("w2t", [CC, 9, NM], F32, kind="ExternalInput")
    b1_d = nc.dram_tensor("b1v", [CC, 1], F32, kind="ExternalInput")
    b2_d = nc.dram_tensor("b2v", [NM, 1], F32, kind="ExternalInput")
    osum_d = nc.dram_tensor("osum", [NM, NQ], F32, kind="ExternalInput")
    orep_d = nc.dram_tensor("orep", [NQ, NM], F32, kind="ExternalInput")
    # gather selectors: sel4[k] is [NM, NQ] with column q = one-hot(q*25+k)
    sel4_d = nc.dram_tensor("sel4", [NM, KA, NQ], F32, kind="ExternalInput")
    # broadcast selectors: selb[q] is [NQ, 128] row-q of ones
    selb_d = nc.dram_tensor("selb", [NQ, NQ, 128], F32, kind="ExternalInput")
    # bf16 copies of the padded input, even- and odd-shifted (for DVE 2x mode
    # alignment: a window starting at odd dj reads the odd-shifted copy at an
    # even element offset)
    xbe_d = [nc.dram_tensor(f"xbe{c}", [128, HP, WP], BF16, kind="ExternalInput")
             for c in range(2)]
    xbo_d = [nc.dram_tensor(f"xbo{c}", [128, HP, WP], BF16, kind="ExternalInput")
             for c in range(2)]

    out_d = nc.dram_tensor("out", [2, 128, NQ, NPIX], F32, kind="ExternalOutput")
    msk_dbg_d = None
    if trace_debug:
        msk_dbg_d = nc.dram_tensor("msk_dbg", [NM, NPIX], F32, kind="ExternalOutput")

    with tile.TileContext(nc) as tc:
        with (
            tc.tile_pool(name="xpool", bufs=1) as xpool,
            tc.tile_pool(name="wpool", bufs=1) as wpool,
            tc.tile_pool(name="mpool", bufs=1) as mpool,
            tc.tile_pool(name="acc", bufs=1) as accpool,
            tc.tile_pool(name="scratch", bufs=2) as scratch,
            tc.tile_pool(name="psum", bufs=2, space="PSUM") as psum,
            tc.tile_pool(name="psum_rep", bufs=4, space="PSUM") as psum_rep,
        ):
            # ---- load inputs -------------------------------------------
            x0 = xpool.tile([128, HP, WP], F32)
            x1 = xpool.tile([128, HP, WP], F32)
            nc.sync.dma_start(x0[:], x0_d[:])
            nc.sync.dma_start(x1[:], x1_d[:])
            xbe0 = xpool.tile([128, HP, WP], BF16, tag="xbe0")
            xbe1 = xpool.tile([128, HP, WP], BF16, tag="xbe1")
            xbo0 = xpool.tile([128, HP, WP], BF16, tag="xbo0")
            xbo1 = xpool.tile([128, HP, WP], BF16, tag="xbo1")
            nc.sync.dma_start(xbe0[:], xbe_d[0][:])
            nc.sync.dma_start(xbe1[:], xbe_d[1][:])
            nc.sync.dma_start(xbo0[:], xbo_d[0][:])
            nc.sync.dma_start(xbo1[:], xbo_d[1][:])

            # partition dim must come first for SBUF: store as [128, 2, CC]
            w1sb = wpool.tile([128, 2, CC], F32, tag="w1sb")
            nc.sync.dma_start(w1sb[:, 0, :], w1t_d[0])
            nc.sync.dma_start(w1sb[:, 1, :], w1t_d[1])

            w2sb = wpool.tile([CC, 9, NM], F32, tag="w2sb")
            nc.sync.dma_start(w2sb[:], w2t_d[:])

            b1c = wpool.tile([CC, 1], F32, tag="b1c")
            nc.sync.dma_start(b1c[:], b1_d[:])
            b2c = wpool.tile([NM, 1], F32, tag="b2c")
            nc.sync.dma_start(b2c[:], b2_d[:])
            osum = wpool.tile([NM, NQ], F32, tag="osum")
            nc.sync.dma_start(osum[:], osum_d[:])
            orep = wpool.tile([NQ, NM], F32, tag="orep")
            nc.sync.dma_start(orep[:], orep_d[:])
            sel4 = wpool.tile([NM, KA, NQ], F32, tag="sel4")
            nc.sync.dma_start(sel4[:], sel4_d[:])
            selb = wpool.tile([NQ, NQ, 128], F32, tag="selb")
            nc.sync.dma_start(selb[:], selb_d[:])

            # ---- PE fences: make PE observe each input-DMA semaphore on a
            # tiny standalone matmul, so real (accumulating) matmuls don't
            # exceed the per-instruction sync-wait limit.
            for fap in (
                x0[:, 0, 0:1], x1[:, 0, 0:1], w1sb[:, 0, 0:1],
                w2sb[:, 0, 0:1], osum[:, 0:1], orep[:, 0:1],
                sel4[:, 0, 0:1], selb[:, 0, 0:1],
            ):
                psf = psum.tile([1, 1], F32, tag="psf")
                nc.tensor.matmul(psf[:], fap, fap, start=True, stop=True)

            # ---- stage A: conv1x1  xc[cc, pix'] over the padded grid ----
            xc = mpool.tile([CC, HP, WP], F32, tag="xc")
            xc_flat = xc[:].rearrange("c h w -> c (h w)")
            x0_flat = x0[:].rearrange("c h w -> c (h w)")
            x1_flat = x1[:].rearrange("c h w -> c (h w)")
            CHUNK = 512
            nchunks = (NPADPIX + CHUNK - 1) // CHUNK  # 5 (last = 400)
            for i in range(nchunks):
                n0 = i * CHUNK
                n1 = min(NPADPIX, n0 + CHUNK)
                ps = psum.tile([CC, CHUNK], F32, tag="ps")
                nc.tensor.matmul(
                    ps[:, : n1 - n0], w1sb[:, 0, :], x0_flat[:, n0:n1],
                    start=True, stop=False,
                )
                nc.tensor.matmul(
                    ps[:, : n1 - n0], w1sb[:, 1, :], x1_flat[:, n0:n1],
                    start=False, stop=True,
                )
                # += b1 while copying PSUM -> SBUF
                nc.vector.tensor_scalar_add(
                    xc_flat[:, n0:n1], ps[:, : n1 - n0], b1c[:, 0:1]
                )

            # ---- stage B: conv3x3 -> mask_raw, fused exp((.)+b2) -------
            # output pixels: h in 0..31 (padded row h+2), w in 0..63 (padded col w+2)
            msk_e = mpool.tile([NM, NPIX], F32, tag="msk_e")  # exp(mask_raw)
            HROWS = 8  # rows per 512-chunk
            for i in range(HL // HROWS):  # 4 chunks
                psm = psum.tile([NM, HROWS, W], F32, tag="ps")
                for tap in range(9):
                    dy, dx = tap // 3, tap % 3
                    rhs = xc[:, i * HROWS + 1 + dy : i * HROWS + 1 + dy + HROWS,
                             1 + dx : 1 + dx + W]
                    nc.tensor.matmul(
                        psm[:], w2sb[:, tap, :], rhs,
                        start=(tap == 0), stop=(tap == 8),
                    )
                me = msk_e[:].rearrange("m (h w) -> m h w", w=W)
                nc.scalar.activation(
                    me[:, i * HROWS : (i + 1) * HROWS, :], psm[:],
                    mybir.ActivationFunctionType.Exp, bias=b2c[:, 0:1],
                )

            # ---- stage C: softmax denominators + normalize -------------
            rs = mpool.tile([NQ, NPIX], F32, tag="rs")  # 1/sum per (q, pix)
            for i in range(NPIX // CHUNK):  # 4
                pss = psum.tile([NQ, CHUNK], F32, tag="ps")
                nc.tensor.matmul(
                    pss[:], osum[:], msk_e[:, i * CHUNK : (i + 1) * CHUNK],
                    start=True, stop=True,
                )
                nc.vector.reciprocal(rs[:, i * CHUNK : (i + 1) * CHUNK], pss[:])

            msk_n = mpool.tile([NM, NPIX], F32, tag="msk_n")
            for i in range(NPIX // CHUNK):
                psr = psum.tile([NM, CHUNK], F32, tag="ps")
                nc.tensor.matmul(
                    psr[:], orep[:], rs[:, i * CHUNK : (i + 1) * CHUNK],
                    start=True, stop=True,
                )
                nc.vector.tensor_mul(
                    msk_n[:, i * CHUNK : (i + 1) * CHUNK],
                    msk_e[:, i * CHUNK : (i + 1) * CHUNK], psr[:],
                )

            if trace_debug:
                nc.sync.dma_start(msk_dbg_d[:], msk_n[:])

            # ---- stage D1: combine (correctness-first) -----------------
            # acc[ch][c, q, pix] += msk_n[q*25+k, pix] * x[ch][c, window_k]
            acc0 = accpool.tile([128, NQ, NPIX], F32, tag="acc0")
            acc1 = accpool.tile([128, NQ, NPIX], F32, tag="acc1")
            nc.vector.memset(acc0[:], 0.0)
            nc.gpsimd.memset(acc1[:], 0.0)

            nadds = 0
            xbe = (xbe0, xbe1)
            xbo = (xbo0, xbo1)
            accs = (acc0, acc1)
            for k in range(KA):
                di, dj = k // 5, k % 5
                # pick the x copy whose window start is 4B-aligned in bf16
                xw, djw = (xbe, dj) if dj % 2 == 0 else (xbo, dj - 1)
                # stage 1: gather the 4 q-rows of tap k to partitions 0..3
                m4 = scratch.tile([NQ, NPIX], F32, tag="m4")
                for i in range(NPIX // CHUNK):
                    p4 = psum.tile([NQ, CHUNK], F32, tag="ps")
                    nc.tensor.matmul(
                        p4[:], sel4[:, k, :],
                        msk_n[:, i * CHUNK : (i + 1) * CHUNK],
                        start=True, stop=True,
                    )
                    nc.scalar.copy(m4[:, i * CHUNK : (i + 1) * CHUNK], p4[:])
                for q in range(NQ):
                    # stage 2: broadcast row q of m4 across 128 partitions
                    # (PE), cast to bf16 (ACT), multiply vs x-window (DVE
                    # 2x bf16), accumulate into fp32 acc (DVE/GPSIMD).
                    prod0 = scratch.tile([128, HL, W], BF16, tag="prod0")
                    prod1 = scratch.tile([128, HL, W], BF16, tag="prod1")
                    prods = [prod0, prod1]
                    prepb = scratch.tile([128, NPIX], BF16, tag="prepb")
                    for i in range(NPIX // CHUNK):
                        prep = psum_rep.tile([128, CHUNK], F32, tag="prep")
                        nc.tensor.matmul(
                            prep[:],
                            selb[:, q, :],
                            m4[:, i * CHUNK : (i + 1) * CHUNK],
                            start=True, stop=True,
                        )
                        nc.scalar.copy(
                            prepb[:, i * CHUNK : (i + 1) * CHUNK], prep[:]
                        )
                    prepv = prepb[:].rearrange("c (h w) -> c h w", w=W)
                    for ch in range(2):
                        xwin = xw[ch][:, di : di + HL, djw : djw + W]
                        nc.vector.tensor_mul(prods[ch][:], xwin, prepv)
                    for ch in range(2):
                        accv = accs[ch][:].rearrange("c q (h w) -> c q h w", w=W)
                        # split the adds between DVE and GPSIMD (~2:1)
                        eng = nc.gpsimd if (nadds % 2 == 0) else nc.vector
                        nadds += 1
                        eng.tensor_add(accv[:, q], accv[:, q], prods[ch][:])

            # ---- write out ---------------------------------------------
            nc.sync.dma_start(out_d[0], acc0[:])
            nc.sync.dma_start(out_d[1], acc1[:])

    nc.compile()
    return nc


@lru_cache(maxsize=2)
def _get_program(trace_debug: bool = False):
    return _build_program(trace_debug)


def _host_prep(x, w1, b1, w2, b2):
    """Build per-core input maps."""
    x = np.asarray(x, np.float32)
    w1 = np.asarray(w1, np.float32)
    b1 = np.asarray(b1, np.float32).reshape(CC, 1)
    w2 = np.asarray(w2, np.float32)
    b2 = np.asarray(b2, np.float32).reshape(NM, 1)

    w1t = np.ascontiguousarray(
        w1[:, :, 0, 0].T.reshape(2, 128, CC)
    )  # [c-tile, 128, CC]
    # w2: (100, 64, 3, 3) -> [cc, tap, m]
    w2t = np.ascontiguousarray(w2.transpose(1, 2, 3, 0).reshape(CC, 9, NM))
    osum = np.zeros((NM, NQ), np.float32)
    for q in range(NQ):
        osum[q * KA : (q + 1) * KA, q] = 1.0
    orep = np.ascontiguousarray(osum.T)
    sel4 = np.zeros((NM, KA, NQ), np.float32)
    for k in range(KA):
        for q in range(NQ):
            sel4[q * KA + k, k, q] = 1.0
    selb = np.zeros((NQ, NQ, 128), np.float32)
    for q in range(NQ):
        selb[q, q, :] = 1.0

    in_maps = []
    for s in range(N_CORES):
        b, hh = s // 2, s % 2
        h0 = hh * HL
        xpad = np.zeros((C, HP, WP), np.float32)
        r0 = max(0, h0 - 2)
        r1 = min(H, h0 + HL + 2)
        xpad[:, (r0 - h0 + 2) : (r1 - h0 + 2), 2 : 2 + W] = x[b, :, r0:r1, :]
        xb = xpad.astype(_BF16NP)
        xbo = np.zeros_like(xb)
        xbo[:, :, :-1] = xb[:, :, 1:]
        in_maps.append(
            {
                "x0": np.ascontiguousarray(xpad[:128]),
                "x1": np.ascontiguousarray(xpad[128:]),
                "xbe0": np.ascontiguousarray(xb[:128]),
                "xbe1": np.ascontiguousarray(xb[128:]),
                "xbo0": np.ascontiguousarray(xbo[:128]),
                "xbo1": np.ascontiguousarray(xbo[128:]),
                "w1t": w1t,
                "w2t": w2t,
                "b1v": b1,
                "b2v": b2,
                "osum": osum,
                "orep": orep,
                "sel4": sel4,
                "selb": selb,
            }
        )
    return in_maps


def _host_post(results):
    """Reassemble full output from per-core results."""
    out = np.empty((B, C, H * SF, W * SF), np.float32)
    for s in range(N_CORES):
        b, hh = s // 2, s % 2
        o = results[s]["out"]  # [2, 128, NQ, NPIX]
        o = o.reshape(2, 128, NQ, HL, W).reshape(C, SF, SF, HL, W)
        # out[c, 2h+sh, 2w+sw] = o[c, sh, sw, h, w]
        o = o.transpose(0, 3, 1, 4, 2).reshape(C, HL * SF, W * SF)
        out[b, :, hh * HL * SF : (hh + 1) * HL * SF, :] = o
    return out


def kernel(x, w1, b1, w2, b2):
    nc = _get_program(bool(int(os.environ.get("CARAFE_DEBUG", "0"))))
    in_maps = _host_prep(x, w1, b1, w2, b2)
    res = run_bass_kernel_spmd(nc, in_maps, list(range(N_CORES)))
    return _host_post(res.results)



# revision 12
# speedup vs baseline: 2.5628x; 2.5628x over previous
"""CARAFE upsampling kernel for 8 Trainium2 NeuronCores — PE-centric design.

Reference op (per batch b):
  xc   = conv1x1(x, w1) + b1                     # (CC=64, H, W)
  mask = conv3x3(xc, w2, pad=1) + b2             # (100, H, W)
  mask = softmax over the 25 kernel taps (per q in 4 = SF*SF groups)
  out[q, c, h, w] = sum_k mask[q, k, h, w] * x[c, h+di-2, w+dj-2]
  out pixel-shuffled by SF=2 -> (C, 2H, 2W)

Sharding: 8 shards = batch(4) x H-halves(2); each core computes 32
output rows from a padded x slice (2-row halo).

Reassembly: pixels are grouped 4-at-a-time (same row). Each group's
25-tap weighted gather over all 256 channels is ONE tensor-engine
matmul: out[(i,q), c] = sum_{(i,k)} stat[(i,k),(i,q)] * xunf[(i,k), c],
with stat a block-diagonal mask stationary [128, 32] (row blocks of 32:
25 taps + 7 zeros; cols 16..31 zero so PSUM col-tiles sit at offsets
{0,32,64,96}) and xunf [128, 256] DMA-gathered from a host-prepared
25-replica transposed x. Mask stationaries are routed into place by
one-hot gather matmuls (4 accumulating i-passes per q) plus 16
strided-free PSUM->SBUF copies.
"""

import os
from functools import lru_cache

import numpy as np
import ml_dtypes as _mld

import concourse.bass as bass
import concourse.mybir as mybir
from concourse import bacc
import concourse.tile as tile
from concourse.ap import AP
from concourse.bass_utils import run_bass_kernel_spmd

F32 = mybir.dt.float32
BF16 = mybir.dt.bfloat16
_BF = _mld.bfloat16

# Problem constants (hardcoded; kernel.py must be self-contained).
B, C, H, W = 4, 256, 64, 64
CC = 64           # compressed channels
SF = 2            # scale factor
K5 = 5            # up-kernel
KA = K5 * K5      # kernel_area = 25
NQ = SF * SF      # 4 quadrants
NM = NQ * KA      # 100 mask channels, original order m = q*25 + k

HL = 32           # local (per-shard) output rows
HP = HL + 4       # padded rows
WP = W + 4        # padded cols
NPIX = HL * W     # 2048 output pixels per shard
NPADPIX = HP * WP # 2448 padded pixels

G = 4             # pixels per matmul group (same row)
SC = 32           # stationary cols per group (16 used + 16 zero pad)
NG = NPIX // G    # 512 groups
NT = 16           # pixel tiles (2 rows = 128 pix = 32 groups each)
GPT = NG // NT    # 32 groups per tile

N_CORES = 8


def _build_program(ntiles: int = NT):
    """Build the SPMD Bass program (identical on all cores)."""
    nc = bacc.Bacc("TRN2", target_bir_lowering=False, debug=False)

    # ---- DRAM parameters -------------------------------------------------
    x0_d = nc.dram_tensor("x0", [128, HP, WP], F32, kind="ExternalInput")
    x1_d = nc.dram_tensor("x1", [128, HP, WP], F32, kind="ExternalInput")
    xt25_d = nc.dram_tensor("xt25", [KA, HP, WP, C], BF16, kind="ExternalInput")
    w1t_d = nc.dram_tensor("w1t", [2, 128, CC], F32, kind="ExternalInput")
    w2t_d = nc.dram_tensor("w2t", [CC, 9, NM], F32, kind="ExternalInput")
    b1_d = nc.dram_tensor("b1v", [CC, 1], F32, kind="ExternalInput")
    b2_d = nc.dram_tensor("b2v", [NM, 1], F32, kind="ExternalInput")
    osum_d = nc.dram_tensor("osum", [NM, NQ], F32, kind="ExternalInput")
    orep_d = nc.dram_tensor("orep", [NQ, NM], F32, kind="ExternalInput")
    sel_d = nc.dram_tensor("sel", [NM, G * NQ, 128], BF16, kind="ExternalInput")
    # out rows r = (gl%4)*32 + i*4 + q (+16..31 zeros); dma d = t*4 + gl//8
    out_d = nc.dram_tensor("out", [NT * 4, 128, 2 * C], BF16, kind="ExternalOutput")

    with tile.TileContext(nc) as tc:
        with (
            tc.tile_pool(name="xpool", bufs=1) as xpool,
            tc.tile_pool(name="wpool", bufs=1) as wpool,
            tc.tile_pool(name="mpool", bufs=1) as mpool,
            tc.tile_pool(name="unf", bufs=3) as unf,
            tc.tile_pool(name="stg", bufs=3) as stgp,
            tc.tile_pool(name="psum", bufs=2, space="PSUM") as psum,
            tc.tile_pool(name="psQ", bufs=1, space="PSUM") as psQ,
            tc.tile_pool(name="psD", bufs=3, space="PSUM") as psD,
        ):
            # ---- load inputs -------------------------------------------
            x0 = xpool.tile([128, HP, WP], F32)
            x1 = xpool.tile([128, HP, WP], F32)
            nc.sync.dma_start(x0[:], x0_d[:])
            nc.sync.dma_start(x1[:], x1_d[:])

            w1sb = wpool.tile([128, 2, CC], F32, tag="w1sb")
            nc.sync.dma_start(w1sb[:, 0, :], w1t_d[0])
            nc.sync.dma_start(w1sb[:, 1, :], w1t_d[1])

            w2sb = wpool.tile([CC, 9, NM], F32, tag="w2sb")
            nc.sync.dma_start(w2sb[:], w2t_d[:])

            b1c = wpool.tile([CC, 1], F32, tag="b1c")
            nc.sync.dma_start(b1c[:], b1_d[:])
            b2c = wpool.tile([NM, 1], F32, tag="b2c")
            nc.sync.dma_start(b2c[:], b2_d[:])
            osum = wpool.tile([NM, NQ], F32, tag="osum")
            nc.sync.dma_start(osum[:], osum_d[:])
            orep = wpool.tile([NQ, NM], F32, tag="orep")
            nc.sync.dma_start(orep[:], orep_d[:])
            sel = wpool.tile([NM, G * NQ, 128], BF16, tag="sel")
            nc.sync.dma_start(sel[:], sel_d[:])

            stat = mpool.tile([128, NG * SC], BF16, tag="stat")
            FE_s = NG * SC
            nc.gpsimd.memset(stat[:], 0.0)

            # ---- PE fences: make PE observe each input-DMA semaphore on a
            # tiny standalone matmul, so real (accumulating) matmuls don't
            # exceed the per-instruction sync-wait limit.
            for fap in (
                x0[:, 0, 0:1], x1[:, 0, 0:1], w1sb[:, 0, 0:1],
                w2sb[:, 0, 0:1], osum[:, 0:1], orep[:, 0:1],
                sel[:, 0, 0:1],
            ):
                psf = psum.tile([1, 1], F32, tag="psf")
                nc.tensor.matmul(psf[:], fap, fap, start=True, stop=True)

            # ---- stage A: conv1x1  xc[cc, pix'] over the padded grid ----
            xc = mpool.tile([CC, HP, WP], F32, tag="xc")
            xc_flat = xc[:].rearrange("c h w -> c (h w)")
            x0_flat = x0[:].rearrange("c h w -> c (h w)")
            x1_flat = x1[:].rearrange("c h w -> c (h w)")
            CHUNK = 512
            nchunks = (NPADPIX + CHUNK - 1) // CHUNK  # 5 (last = 400)
            for i in range(nchunks):
                n0 = i * CHUNK
                n1 = min(NPADPIX, n0 + CHUNK)
                ps = psum.tile([CC, CHUNK], F32, tag="ps")
                nc.tensor.matmul(
                    ps[:, : n1 - n0], w1sb[:, 0, :], x0_flat[:, n0:n1],
                    start=True, stop=False,
                )
                nc.tensor.matmul(
                    ps[:, : n1 - n0], w1sb[:, 1, :], x1_flat[:, n0:n1],
                    start=False, stop=True,
                )
                # += b1 while copying PSUM -> SBUF
                nc.vector.tensor_scalar_add(
                    xc_flat[:, n0:n1], ps[:, : n1 - n0], b1c[:, 0:1]
                )

            # ---- stage B: conv3x3 -> mask_raw, fused exp((.)+b2) -------
            msk_e = mpool.tile([NM, NPIX], F32, tag="msk_e")  # exp(mask_raw)
            HROWS = 8  # rows per 512-chunk
            for i in range(HL // HROWS):  # 4 chunks
                psm = psum.tile([NM, HROWS, W], F32, tag="ps")
                for tap in range(9):
                    dy, dx = tap // 3, tap % 3
                    rhs = xc[:, i * HROWS + 1 + dy : i * HROWS + 1 + dy + HROWS,
                             1 + dx : 1 + dx + W]
                    nc.tensor.matmul(
                        psm[:], w2sb[:, tap, :], rhs,
                        start=(tap == 0), stop=(tap == 8),
                    )
                me = msk_e[:].rearrange("m (h w) -> m h w", w=W)
                nc.scalar.activation(
                    me[:, i * HROWS : (i + 1) * HROWS, :], psm[:],
                    mybir.ActivationFunctionType.Exp, bias=b2c[:, 0:1],
                )

            # ---- stage C: softmax denominators + normalize -------------
            rs = mpool.tile([NQ, NPIX], F32, tag="rs")  # 1/sum per (q, pix)
            for i in range(NPIX // CHUNK):  # 4
                pss = psum.tile([NQ, CHUNK], F32, tag="ps")
                nc.tensor.matmul(
                    pss[:], osum[:], msk_e[:, i * CHUNK : (i + 1) * CHUNK],
                    start=True, stop=True,
                )
                nc.vector.reciprocal(rs[:, i * CHUNK : (i + 1) * CHUNK], pss[:])

            msk_n = mpool.tile([NM, NPIX], BF16, tag="msk_n")
            for i in range(NPIX // CHUNK):
                psr = psum.tile([NM, CHUNK], F32, tag="ps")
                nc.tensor.matmul(
                    psr[:], orep[:], rs[:, i * CHUNK : (i + 1) * CHUNK],
                    start=True, stop=True,
                )
                nc.vector.tensor_mul(
                    msk_n[:, i * CHUNK : (i + 1) * CHUNK],
                    msk_e[:, i * CHUNK : (i + 1) * CHUNK], psr[:],
                )

            # ---- stage S: build block-diag stationaries ----------------
            # stat[(i*32+k), g*SC + i*4 + q] = msk_n[q*25+k, 4g+i]
            mska = msk_n[:]
            stata = stat[:]
            FE_m = NPIX
            for q in range(NQ):
                psq = psQ.tile([128, NG], F32, tag="psq")
                for i in range(G):
                    rhs = AP(mska.tensor, mska.offset + i,
                             [[FE_m, NM], [G, NG]])
                    nc.tensor.matmul(
                        psq[:], sel[:, i * NQ + q, :], rhs,
                        start=(i == 0), stop=(i == G - 1),
                    )
                for i in range(G):
                    dst = AP(stata.tensor,
                             stata.offset + (i * 32) * FE_s + i * NQ + q,
                             [[FE_s, 32], [SC, NG]])
                    src = psq[i * 32 : (i + 1) * 32, :]
                    if (i + q) % 2 == 0:
                        nc.scalar.copy(dst, src)
                    else:
                        nc.vector.tensor_scalar_add(dst, src, 0.0)

            # PE fence on stat (observe the 16 copies once)
            psf = psum.tile([1, 1], F32, tag="psf")
            nc.tensor.matmul(psf[:], stat[:, 0:1], stat[:, 0:1],
                             start=True, stop=True)

            # ---- stage D: per-tile unfold DMA + group matmuls ----------
            XFE = GPT * C  # x_unf tile free extent
            for t in range(ntiles):
                xu = unf.tile([128, GPT, C], BF16, tag="xu")
                if t < 3:
                    # first use of each rotating buffer: clear so the 7
                    # dead rows per 32-block can't hold NaN garbage
                    nc.gpsimd.memset(xu[:], 0.0)
                xua = xu[:]
                for i in range(G):
                    for gh in range(2):
                        dst = AP(xua.tensor,
                                 xua.offset + (i * 32) * XFE + gh * 16 * C,
                                 [[XFE, KA], [C, 16], [1, C]])
                        src = AP(xt25_d[:].tensor,
                                 ((t * 2 + gh) * WP + i) * C,
                                 [[HP * WP * C, KA], [G * C, 16], [1, C]])
                        eng = nc.sync if i % 2 == 0 else nc.scalar
                        eng.dma_start(dst, src)
                # PE fence for this tile's unfold DMAs
                psf = psum.tile([1, 1], F32, tag="psf")
                nc.tensor.matmul(psf[:], xu[:, 0, 0:1], xu[:, 0, 0:1],
                                 start=True, stop=True)

                for half in range(GPT // 8):  # 4 out-DMA batches per tile
                    ostg = stgp.tile([128, 2, C], BF16, tag="ostg")
                    for ph in range(2):      # 2 psum tiles per batch
                        ps = psD.tile([128, C], F32, tag="psd")
                        for r in range(4):   # 4 groups per psum tile
                            gl = half * 8 + ph * 4 + r
                            g = t * GPT + gl
                            nc.tensor.matmul(
                                ps[r * 32 : (r + 1) * 32, :],
                                stat[:, g * SC : (g + 1) * SC],
                                xu[:, gl, :],
                                start=True, stop=True,
                                tile_position=(0, r * 32),
                            )
                        if ph == 0:
                            nc.vector.tensor_scalar_add(
                                ostg[:, ph, :], ps[:], 0.0)
                        else:
                            nc.scalar.copy(ostg[:, ph, :], ps[:])
                    nc.sync.dma_start(
                        out_d[t * 4 + half],
                        ostg[:].rearrange("p a c -> p (a c)"),
                    )

    nc.compile()
    return nc


@lru_cache(maxsize=2)
def _get_program():
    return _build_program(int(os.environ.get("CARAFE_NT", str(NT))))


def _host_prep(x, w1, b1, w2, b2):
    """Build per-core input maps."""
    x = np.asarray(x, np.float32)
    w1 = np.asarray(w1, np.float32)
    b1 = np.asarray(b1, np.float32).reshape(CC, 1)
    w2 = np.asarray(w2, np.float32)
    b2 = np.asarray(b2, np.float32).reshape(NM, 1)

    w1t = np.ascontiguousarray(w1[:, :, 0, 0].T.reshape(2, 128, CC))
    w2t = np.ascontiguousarray(w2.transpose(1, 2, 3, 0).reshape(CC, 9, NM))
    osum = np.zeros((NM, NQ), np.float32)
    for q in range(NQ):
        osum[q * KA : (q + 1) * KA, q] = 1.0
    orep = np.ascontiguousarray(osum.T)
    # one-hot gather: route mask row q*25+k to stationary row i*32+k in
    # the (i, q) pass; it lands at column i*4+q of each group's block.
    sel = np.zeros((NM, G * NQ, 128), dtype=_BF)
    for i in range(G):
        for q in range(NQ):
            for k in range(KA):
                sel[q * KA + k, i * NQ + q, i * 32 + k] = 1.0

    in_maps = []
    for s in range(N_CORES):
        b, hh = s // 2, s % 2
        h0 = hh * HL
        xpad = np.zeros((C, HP, WP), np.float32)
        r0 = max(0, h0 - 2)
        r1 = min(H, h0 + HL + 2)
        xpad[:, (r0 - h0 + 2) : (r1 - h0 + 2), 2 : 2 + W] = x[b, :, r0:r1, :]
        xpT = np.ascontiguousarray(xpad.transpose(1, 2, 0)).astype(_BF)
        xt25 = np.zeros((KA, HP, WP, C), _BF)
        for k in range(KA):
            di, dj = k // K5, k % K5
            xt25[k, : HP - di, : WP - dj] = xpT[di:, dj:]
        in_maps.append(
            {
                "x0": np.ascontiguousarray(xpad[:128]),
                "x1": np.ascontiguousarray(xpad[128:]),
                "xt25": xt25,
                "w1t": w1t,
                "w2t": w2t,
                "b1v": b1,
                "b2v": b2,
                "osum": osum,
                "orep": orep,
                "sel": sel,
            }
        )
    return in_maps


def _host_post(results):
    """Reassemble full output from per-core results."""
    out = np.empty((B, C, H * SF, W * SF), np.float32)
    for s in range(N_CORES):
        b, hh = s // 2, s % 2
        o = results[s]["out"]  # [64, 128, 512] bf16
        # dims: d = t*4 + dq ; r = rblk*32 + sub ; f = ph*256 + c
        # gl = dq*8 + ph*4 + rblk ; sub = i*4 + q (sub < 16)
        o = np.asarray(o).reshape(NT, 4, 4, 32, 2, C)  # t, dq, rblk, sub, ph, c
        o = o[:, :, :, :16]                     # drop zero rows
        o = o.transpose(0, 1, 4, 2, 3, 5)       # t, dq, ph, rblk, sub, c
        o = o.reshape(NG, G, NQ, C)             # g, i, q, c
        o = o.reshape(HL, W // G, G, NQ, C)     # h, gw, i, q, c
        o = o.transpose(4, 0, 1, 2, 3).reshape(C, HL, W, SF, SF)
        # out[c, 2h+sh, 2w+sw] = o[c, h, w, sh, sw]
        o = o.transpose(0, 1, 3, 2, 4).reshape(C, HL * SF, W * SF)
        out[b, :, hh * HL * SF : (hh + 1) * HL * SF, :] = o.astype(np.float32)
    return out


def kernel(x, w1, b1, w2, b2):
    nc = _get_program()
    in_maps = _host_prep(x, w1, b1, w2, b2)
    res = run_bass_kernel_spmd(nc, in_maps, list(range(N_CORES)))
    return _host_post(res.results)
